# revision 14
# baseline (speedup 1.0000x reference)
"""Trainium2 Bass kernel for nn_CDSPMoELayer (task-conditioned dual-subspace MoE).

Math reformulation (verified bit-close to the reference on CPU):
  Since Wd[e,r,:] = W_down[:, tl_idx[e,r]] and Wu[e,r,:] = W_up[tl_idx[e,r], :],
  the per-expert low-rank einsums collapse to dense matmuls:
      H = x @ W_down            [N, DB]
      G = gelu_tanh(H)
      C = P @ Mg                [N, DB]   (P: top-2 routing weights scattered
                                           over E; Mg[e,j] = gate[e]*mask[e,j])
      y = (G * C) @ W_up        [N, D]
  The router logits need only two per-batch scalars from the layernorm:
      logits = rs_b * (x @ Wr[:D]) + (tb_br[b] - rs_b*mu_b*colsum(Wr[:D]))

Sharding: data-parallel over tokens. 8 cores; cores 0-3 take batch 0,
cores 4-7 batch 1; each core handles 1024 tokens. Per-batch layernorm
stats (sum, sumsq) are exchanged with a 16-byte AllReduce.

Precision: H/C/y matmuls in bf16 (weights host-cast to bf16); the router
matmul q = x @ Wr1 stays full fp32 so top-2 selections match the reference.
"""

import sys

sys.path.insert(0, "/opt/trn_rl_repo")

import numpy as np
import ml_dtypes

import concourse.bass as bass
import concourse.mybir as mybir
import concourse.tile as tile_mod
import concourse.bass_isa as bass_isa
from concourse.masks import make_identity

# ---------------------------------------------------------------- problem dims
P = 128
B, S, D = 2, 4096, 1024
E, DB, DT = 16, 256, 32
RANK_QUOTA = 64
EPS = 1e-5
NCORES = 8
TOK = B * S // NCORES          # tokens per core = 1024
CORES_PER_BATCH = NCORES // B  # 4
NTOT = S * D                   # stats denominator per batch = 2^22
PER_PART = TOK * D // P        # elements per sbuf partition of the x shard

F32 = mybir.dt.float32
BF16 = mybir.dt.bfloat16

GELU_FUNC = mybir.ActivationFunctionType.Gelu_apprx_tanh


# ------------------------------------------------------- walrus wait workaround
# This container's walrus rejects instructions carrying more than one sem wait
# ("Too many sync wait commands").  Tile's wait assigner can attach several.
# Post-process the serialized BIR: move excess waits onto preceding Drain
# instructions on the same engine, one wait each.
def _split_excess_waits(m):
    n = 0
    for f in m["functions"]:
        blocks = f.get("basicblocks") or f.get("blocks") or []
        for blk in blocks:
            out = []
            for inst in blk["instructions"]:
                si = inst.get("sync_info")
                ow = si.get("on_wait") if si else None
                if ow and len(ow) > 1:
                    for w in ow[:-1]:
                        n += 1
                        out.append(
                            {
                                "debug": inst.get("debug"),
                                "engine": inst["engine"],
                                "ins": [],
                                "outs": [],
                                "name": f"I-wsplit-{n}",
                                "opcode": "Drain",
                                "sync_info": {"on_update": [], "on_wait": [w]},
                            }
                        )
                    si["on_wait"] = [ow[-1]]
                out.append(inst)
            blk["instructions"] = out
    return n


_orig_to_json_bytes = bass.Bass.to_json_bytes


def _patched_to_json_bytes(self):
    import orjson

    raw = _orig_to_json_bytes(self)
    m = orjson.loads(raw)
    if _split_excess_waits(m):
        return orjson.dumps(m)
    return raw


bass.Bass.to_json_bytes = _patched_to_json_bytes


# ------------------------------------------------------------------ the kernel
def build_nc():
    nc = bass.Bass()
    AF = mybir.ActivationFunctionType
    ALU = mybir.AluOpType
    AX = mybir.AxisListType

    x_h = nc.dram_tensor("x", [TOK, D], F32, kind="ExternalInput")
    wd_h = nc.dram_tensor("wd", [D, DB], BF16, kind="ExternalInput")
    wu_h = nc.dram_tensor("wu", [DB, D], BF16, kind="ExternalInput")
    wr1_h = nc.dram_tensor("wr1", [D, E], F32, kind="ExternalInput")
    mg_h = nc.dram_tensor("mg", [E, DB], BF16, kind="ExternalInput")
    tbbr_h = nc.dram_tensor("tbbr", [E], F32, kind="ExternalInput")
    colsum_h = nc.dram_tensor("colsum", [E], F32, kind="ExternalInput")
    bsel_h = nc.dram_tensor("bsel", [B], F32, kind="ExternalInput")
    y_h = nc.dram_tensor("y", [TOK, D], F32, kind="ExternalOutput")

    cc_in = nc.dram_tensor("cc_in", [B, 2], F32)
    cc_out = nc.dram_tensor("cc_out", [B, 2], F32, addr_space="Shared")

    DC = D // P     # 8 d-chunks
    NT = TOK // P   # 8 token tiles
    NC512 = TOK // 512  # 2 chunks of 512 tokens
    JM = DB // P    # 2 DB chunks

    with tile_mod.TileContext(nc) as tc:
        with (
            tc.tile_pool(name="big", bufs=1) as big,
            tc.tile_pool(name="consts", bufs=1) as consts,
            tc.tile_pool(name="small", bufs=1) as small,
            tc.tile_pool(name="route", bufs=1) as route,
            tc.tile_pool(name="ysb", bufs=3) as ysb_pool,
            # PSUM budget (8 banks): tr 3 + mm 4 + q 1
            tc.tile_pool(name="psTR", bufs=3, space="PSUM") as psTR,
            tc.tile_pool(name="psMM", bufs=4, space="PSUM") as psMM,
            tc.tile_pool(name="psQ", bufs=1, space="PSUM") as psQ,
        ):
            # ---------------- constants / weights
            ident = consts.tile([P, P], F32)
            make_identity(nc, ident[:])

            ones1x128 = consts.tile([1, P], F32)
            nc.vector.memset(ones1x128[:], 1.0)
            ones128 = consts.tile([P, 1], F32)
            nc.vector.memset(ones128[:], 1.0)
            eps_row = consts.tile([1, 1], F32)
            nc.vector.memset(eps_row[:], EPS)

            wr1_sb = consts.tile([P, DC, E], F32)
            nc.sync.dma_start(
                out=wr1_sb[:], in_=wr1_h[:, :].rearrange("(dc p) e -> p dc e", p=P)
            )
            mg_sb = consts.tile([E, DB], BF16)
            nc.sync.dma_start(out=mg_sb[:], in_=mg_h[:, :])

            tbbr_row = consts.tile([1, E], F32)
            nc.sync.dma_start(out=tbbr_row[:], in_=tbbr_h[:][None, :])
            colsum_row = consts.tile([1, E], F32)
            nc.sync.dma_start(out=colsum_row[:], in_=colsum_h[:][None, :])
            sel_row = consts.tile([1, B], F32)
            nc.sync.dma_start(out=sel_row[:], in_=bsel_h[:][None, :])

            # ---------------- load x, stats, transpose to [d, tok] layout
            x_sb = big.tile([P, NT, D], F32)
            xt_f32 = big.tile([P, DC, TOK], F32)
            xt_bf = big.tile([P, DC, TOK], BF16)
            stats_sb = small.tile([P, NT * 2, 6], F32)
            ps_q = psQ.tile([P, NT, E], F32, tag="q")

            ci = 0
            for t in range(NT):
                nc.sync.dma_start(
                    out=x_sb[:, t, :], in_=x_h[t * P : (t + 1) * P, :]
                )
                nc.vector.bn_stats(
                    out=stats_sb[:, 2 * t, :], in_=x_sb[:, t, 0:512]
                )
                nc.vector.bn_stats(
                    out=stats_sb[:, 2 * t + 1, :], in_=x_sb[:, t, 512:1024]
                )
                for g in range(DC // 4):   # 4 transposed blocks per psum tile
                    ps_tr = psTR.tile([P, 512], F32, tag="tr")
                    for k in range(4):
                        dc = g * 4 + k
                        nc.tensor.transpose(
                            ps_tr[:, k * P : (k + 1) * P],
                            x_sb[:, t, dc * P : (dc + 1) * P],
                            ident[:],
                        )
                    src_v = ps_tr[:].rearrange("p (k c) -> p k c", k=4)
                    for dst in (
                        xt_f32[:, g * 4 : (g + 1) * 4, t * P : (t + 1) * P],
                        xt_bf[:, g * 4 : (g + 1) * 4, t * P : (t + 1) * P],
                    ):
                        if ci % 2 == 0:
                            nc.scalar.copy(out=dst, in_=src_v)
                        else:
                            nc.vector.tensor_copy(out=dst, in_=src_v)
                        ci += 1

            wd_sb = consts.tile([P, DC, DB], BF16)
            nc.sync.dma_start(
                out=wd_sb[:], in_=wd_h[:, :].rearrange("(dc p) j -> p dc j", p=P)
            )
            wu_sb = consts.tile([P, JM, D], BF16)
            nc.sync.dma_start(
                out=wu_sb[:], in_=wu_h[:, :].rearrange("(jc p) d -> p jc d", p=P)
            )

            # router matmul q = x @ Wr1 in fp32, x as the stationary operand:
            # output is token-major [tok, E]; emitted after the transpose loop
            # so PE never stalls mid-tile waiting for xt_f32 copies
            for t in range(NT):
                for dc in range(DC):
                    nc.tensor.matmul(
                        ps_q[:, t, :],
                        xt_f32[:, dc, t * P : (t + 1) * P],
                        wr1_sb[:, dc, :],
                        start=(dc == 0),
                        stop=(dc == DC - 1),
                    )

            # per-partition mean/var over this core's shard -> (S1, SS) partials
            mv = small.tile([P, 2], F32)
            nc.vector.bn_aggr(out=mv[:], in_=stats_sb[:])
            s1ss = small.tile([P, 2], F32)
            nc.vector.tensor_scalar_mul(s1ss[:, 0:1], mv[:, 0:1], float(PER_PART))
            msq = small.tile([P, 1], F32)
            nc.vector.tensor_mul(msq[:], mv[:, 0:1], mv[:, 0:1])
            nc.vector.tensor_add(msq[:], msq[:], mv[:, 1:2])
            nc.vector.tensor_scalar_mul(s1ss[:, 1:2], msq[:], float(PER_PART))

            # cross-partition totals + batch scatter, entirely on gpsimd so
            # the collective is not queued behind PE/DVE bulk work
            s_row01 = small.tile([1, 2], F32)
            nc.gpsimd.tensor_reduce(
                s_row01[:], s1ss[:], axis=AX.C, op=ALU.add
            )
            cc_row = small.tile([1, 2 * B], F32)
            nc.gpsimd.tensor_scalar_mul(
                cc_row[:, 0:2], s_row01[:], sel_row[:, 0:1]
            )
            nc.gpsimd.tensor_scalar_mul(
                cc_row[:, 2:4], s_row01[:], sel_row[:, 1:2]
            )
            nc.gpsimd.dma_start(
                out=cc_in[:, :].rearrange("b s -> (b s)")[None, :], in_=cc_row[:]
            )

            # 16-byte cross-core AllReduce of the per-batch (S1, SS)
            nc.gpsimd.collective_compute(
                "AllReduce",
                ALU.add,
                replica_groups=[list(range(NCORES))],
                ins=[cc_in[:, :]],
                outs=[cc_out[:, :]],
            )

            # derive scalars on partition 0: mu, rs, cvec (row oriented)
            stats_row = small.tile([1, B, 2], F32)
            nc.gpsimd.dma_start(
                out=stats_row[:], in_=cc_out[:, :].rearrange("b s -> (b s)")[None, :]
            )
            t2 = small.tile([1, B], F32)
            s1_r = small.tile([1, 1], F32)
            ss_r = small.tile([1, 1], F32)
            nc.vector.tensor_mul(t2[:], stats_row[:, :, 0], sel_row[:])
            nc.vector.reduce_sum(s1_r[:], t2[:], axis=AX.X)
            nc.vector.tensor_mul(t2[:], stats_row[:, :, 1], sel_row[:])
            nc.vector.reduce_sum(ss_r[:], t2[:], axis=AX.X)

            mu_r = small.tile([1, 1], F32)
            var_r = small.tile([1, 1], F32)
            rs_r = small.tile([1, 1], F32)
            nc.vector.tensor_scalar_mul(mu_r[:], s1_r[:], 1.0 / NTOT)
            nc.vector.tensor_scalar_mul(var_r[:], ss_r[:], 1.0 / NTOT)
            musq = small.tile([1, 1], F32)
            nc.vector.tensor_mul(musq[:], mu_r[:], mu_r[:])
            nc.vector.tensor_sub(var_r[:], var_r[:], musq[:])
            nc.scalar.activation(
                out=var_r[:], in_=var_r[:], func=AF.Sqrt, bias=eps_row[:], scale=1.0
            )
            nc.vector.reciprocal(out=rs_r[:], in_=var_r[:])  # rs = 1/sqrt(var+eps)
            rm_r = small.tile([1, 1], F32)
            nc.vector.tensor_mul(rm_r[:], rs_r[:], mu_r[:])
            cvec_row = small.tile([1, E], F32)
            nc.vector.tensor_scalar_mul(cvec_row[:], colsum_row[:], rm_r[:])
            nc.vector.tensor_sub(cvec_row[:], tbbr_row[:], cvec_row[:])

            # broadcast rs and cvec across all 128 partitions via PE outer
            ps_rc = psTR.tile([P, P], F32, tag="tr")
            nc.tensor.matmul(
                ps_rc[:, 0:1], ones1x128[:], rs_r[:], start=True, stop=True
            )
            nc.tensor.matmul(
                ps_rc[:, 1 : 1 + E], ones1x128[:], cvec_row[:], start=True, stop=True
            )
            rc_sb = small.tile([P, 1 + E], F32)
            nc.scalar.copy(out=rc_sb[:], in_=ps_rc[:, : 1 + E])

            # ---------------- H^T = W_down^T x^T (+gelu)
            gt_sb = big.tile([P, JM, TOK], BF16)
            for t5 in range(NC512):
                sl = slice(t5 * 512, (t5 + 1) * 512)
                for jm in range(JM):
                    ps_h = psMM.tile([P, 512], F32, tag="mm")
                    for dc in range(DC):
                        nc.tensor.matmul(
                            ps_h[:],
                            wd_sb[:, dc, jm * P : (jm + 1) * P],
                            xt_bf[:, dc, sl],
                            start=(dc == 0),
                            stop=(dc == DC - 1),
                        )
                    nc.scalar.activation(
                        out=gt_sb[:, jm, sl], in_=ps_h[:], func=GELU_FUNC
                    )

            # ---------------- routing: logits (token-major), top-2, P^T
            logit_n = route.tile([P, NT, E], F32)
            nc.vector.tensor_scalar_mul(logit_n[:], ps_q[:], rc_sb[:, 0:1])
            nc.vector.tensor_tensor(
                logit_n[:],
                logit_n[:],
                rc_sb[:, None, 1 : 1 + E].to_broadcast((P, NT, E)),
                ALU.add,
            )

            m1 = route.tile([P, NT, 1], F32)
            eq1 = route.tile([P, NT, E], F32)
            l2 = route.tile([P, NT, E], F32)
            m2 = route.tile([P, NT, 1], F32)
            eq2 = route.tile([P, NT, E], F32)
            w1 = route.tile([P, NT, 1], F32)
            p_n = route.tile([P, NT, E], F32)

            nc.vector.reduce_max(m1[:], logit_n[:], axis=AX.X)
            nc.vector.tensor_tensor(
                eq1[:], logit_n[:], m1[:].to_broadcast(logit_n.shape), ALU.is_equal
            )
            nc.vector.scalar_tensor_tensor(
                out=l2[:], in0=eq1[:], scalar=-1e30, in1=logit_n[:],
                op0=ALU.mult, op1=ALU.add,
            )
            nc.vector.reduce_max(m2[:], l2[:], axis=AX.X)
            nc.vector.tensor_tensor(
                eq2[:], l2[:], m2[:].to_broadcast(l2.shape), ALU.is_equal
            )
            # w1 = sigmoid(m1 - m2); P = eq2 + w1*(eq1 - eq2)
            nc.vector.tensor_sub(w1[:], m2[:], m1[:])
            nc.scalar.activation(
                out=w1[:], in_=w1[:], func=AF.Sigmoid, scale=-1.0
            )
            nc.vector.tensor_sub(p_n[:], eq1[:], eq2[:])
            nc.vector.tensor_tensor(
                p_n[:], p_n[:], w1[:].to_broadcast(p_n.shape), ALU.mult
            )
            nc.vector.tensor_add(p_n[:], p_n[:], eq2[:])

            pT_sb = route.tile([E, TOK], BF16)
            for t in range(NT):
                ps_pt = psTR.tile([P, P], F32, tag="tr")
                nc.tensor.transpose(ps_pt[:E, :], p_n[:, t, :], ident[:])
                nc.scalar.copy(
                    out=pT_sb[:, t * P : (t + 1) * P], in_=ps_pt[:E, :]
                )

            # ---------------- C^T = Mg^T P^T ; Z^T = G^T * C^T ; y = Z^T^T Wu
            zt_sb = big.tile([P, JM, TOK], BF16)
            for jm in range(JM):
                for t5 in range(NC512):
                    sl = slice(t5 * 512, (t5 + 1) * 512)
                    ps_c = psMM.tile([P, 512], F32, tag="mm")
                    nc.tensor.matmul(
                        ps_c[:],
                        mg_sb[:, jm * P : (jm + 1) * P],
                        pT_sb[:, sl],
                        start=True,
                        stop=True,
                    )
                    nc.vector.tensor_tensor(
                        zt_sb[:, jm, sl], gt_sb[:, jm, sl], ps_c[:], ALU.mult
                    )

            for t in range(NT):
                y_sb = ysb_pool.tile([P, D], F32)
                for dh in range(2):
                    ps_y = psMM.tile([P, 512], F32, tag="mm")
                    for jm in range(JM):
                        nc.tensor.matmul(
                            ps_y[:],
                            zt_sb[:, jm, t * P : (t + 1) * P],
                            wu_sb[:, jm, dh * 512 : (dh + 1) * 512],
                            start=(jm == 0),
                            stop=(jm == JM - 1),
                        )
                    dst = y_sb[:, dh * 512 : (dh + 1) * 512]
                    if dh == 0:
                        nc.scalar.copy(out=dst, in_=ps_y[:])
                    else:
                        nc.vector.tensor_copy(out=dst, in_=ps_y[:])
                nc.sync.dma_start(out=y_h[t * P : (t + 1) * P, :], in_=y_sb[:])

    return nc


_NC_CACHE = {}


def _get_nc():
    if "nc" not in _NC_CACHE:
        _NC_CACHE["nc"] = build_nc()
    return _NC_CACHE["nc"]


def make_in_maps(inputs):
    """Host-side prep: small-tensor precompute + per-core sharding."""
    x = np.ascontiguousarray(np.asarray(inputs["x"], dtype=np.float32))
    task_id = np.asarray(inputs["task_id"])
    task_emb = np.asarray(inputs["task_emb"], dtype=np.float32)
    Wr = np.asarray(inputs["Wr"], dtype=np.float32)
    br = np.asarray(inputs["br"], dtype=np.float32)
    W_down = np.asarray(inputs["W_down"], dtype=np.float32)
    W_up = np.asarray(inputs["W_up"], dtype=np.float32)
    topo_logits = np.asarray(inputs["topo_logits"], dtype=np.float32)

    # gated expert->subspace mask from topo_logits (tiny: [16, 256])
    idx = np.argsort(-topo_logits, axis=1)[:, :RANK_QUOTA]
    mask = np.zeros((E, DB), np.float32)
    np.put_along_axis(mask, idx, 1.0, axis=1)
    tl_vals = np.take_along_axis(topo_logits, idx, axis=1)
    gate = (1.0 / (1.0 + np.exp(-tl_vals))).mean(axis=1)
    mg = np.ascontiguousarray(
        (mask * gate[:, None].astype(np.float32)).astype(ml_dtypes.bfloat16)
    )

    Wr1 = np.ascontiguousarray(Wr[:D])
    tb_br = (task_emb[task_id] @ Wr[D:]) + br          # [B, E]
    colsum = Wr1.sum(axis=0)                            # [E]
    wd_bf = np.ascontiguousarray(W_down.astype(ml_dtypes.bfloat16))
    wu_bf = np.ascontiguousarray(W_up.astype(ml_dtypes.bfloat16))

    xf = x.reshape(B * S, D)
    in_maps = []
    for c in range(NCORES):
        b = c // CORES_PER_BATCH
        t0 = c * TOK
        bsel = np.zeros(B, np.float32)
        bsel[b] = 1.0
        in_maps.append(
            {
                "x": np.ascontiguousarray(xf[t0 : t0 + TOK]),
                "wd": wd_bf,
                "wu": wu_bf,
                "wr1": Wr1,
                "mg": mg,
                "tbbr": np.ascontiguousarray(tb_br[b]),
                "colsum": np.ascontiguousarray(colsum),
                "bsel": bsel,
            }
        )
    return in_maps


def run(inputs, trace=False):
    from concourse.bass_utils import run_bass_kernel_spmd

    nc = _get_nc()
    in_maps = make_in_maps(inputs)
    res = run_bass_kernel_spmd(
        nc, in_maps, core_ids=list(range(NCORES)), trace=trace
    )
    y = np.concatenate(
        [res.results[c]["y"] for c in range(NCORES)], axis=0
    ).reshape(B, S, D)
    return y, res


def kernel(**inputs):
    y, _ = run(inputs, trace=False)
    return y


# revision 17
# speedup vs baseline: 1.0786x; 1.0786x over previous
"""Trainium2 Bass kernel for nn_CDSPMoELayer (task-conditioned dual-subspace MoE).

Math reformulation (verified bit-close to the reference on CPU):
  Since Wd[e,r,:] = W_down[:, tl_idx[e,r]] and Wu[e,r,:] = W_up[tl_idx[e,r], :],
  the per-expert low-rank einsums collapse to dense matmuls:
      H = x @ W_down            [N, DB]
      G = gelu_tanh(H)
      C = P @ Mg                [N, DB]   (P: top-2 routing weights scattered
                                           over E; Mg[e,j] = gate[e]*mask[e,j])
      y = (G * C) @ W_up        [N, D]
  The router logits need only two per-batch scalars from the layernorm:
      logits = rs_b * (x @ Wr[:D]) + (tb_br[b] - rs_b*mu_b*colsum(Wr[:D]))

Sharding: data-parallel over tokens. 8 cores; cores 0-3 take batch 0,
cores 4-7 batch 1; each core handles 1024 tokens. Per-batch layernorm
stats (sum, sumsq) are exchanged with a 16-byte AllReduce.

Precision: H/C/y matmuls in bf16 (weights host-cast to bf16); the router
matmul q = x @ Wr1 stays full fp32 so top-2 selections match the reference.
"""

import sys

sys.path.insert(0, "/opt/trn_rl_repo")

import numpy as np
import ml_dtypes

import concourse.bass as bass
import concourse.mybir as mybir
import concourse.tile as tile_mod
import concourse.bass_isa as bass_isa
from concourse.masks import make_identity

# ---------------------------------------------------------------- problem dims
P = 128
B, S, D = 2, 4096, 1024
E, DB, DT = 16, 256, 32
RANK_QUOTA = 64
EPS = 1e-5
NCORES = 8
TOK = B * S // NCORES          # tokens per core = 1024
CORES_PER_BATCH = NCORES // B  # 4
NTOT = S * D                   # stats denominator per batch = 2^22
PER_PART = TOK * D // P        # elements per sbuf partition of the x shard

F32 = mybir.dt.float32
BF16 = mybir.dt.bfloat16

GELU_FUNC = mybir.ActivationFunctionType.Gelu_apprx_tanh


# ------------------------------------------------------- walrus wait workaround
# This container's walrus rejects instructions carrying more than one sem wait
# ("Too many sync wait commands").  Tile's wait assigner can attach several.
# Post-process the serialized BIR: move excess waits onto preceding Drain
# instructions on the same engine, one wait each.
def _split_excess_waits(m):
    n = 0
    for f in m["functions"]:
        blocks = f.get("basicblocks") or f.get("blocks") or []
        for blk in blocks:
            out = []
            for inst in blk["instructions"]:
                si = inst.get("sync_info")
                ow = si.get("on_wait") if si else None
                if ow and len(ow) > 1:
                    for w in ow[:-1]:
                        n += 1
                        out.append(
                            {
                                "debug": inst.get("debug"),
                                "engine": inst["engine"],
                                "ins": [],
                                "outs": [],
                                "name": f"I-wsplit-{n}",
                                "opcode": "Drain",
                                "sync_info": {"on_update": [], "on_wait": [w]},
                            }
                        )
                    si["on_wait"] = [ow[-1]]
                out.append(inst)
            blk["instructions"] = out
    return n


_orig_to_json_bytes = bass.Bass.to_json_bytes


def _patched_to_json_bytes(self):
    import orjson

    raw = _orig_to_json_bytes(self)
    m = orjson.loads(raw)
    if _split_excess_waits(m):
        return orjson.dumps(m)
    return raw


bass.Bass.to_json_bytes = _patched_to_json_bytes


# ------------------------------------------------------------------ the kernel
def build_nc():
    nc = bass.Bass()
    AF = mybir.ActivationFunctionType
    ALU = mybir.AluOpType
    AX = mybir.AxisListType

    x_h = nc.dram_tensor("x", [TOK, D], F32, kind="ExternalInput")
    wd_h = nc.dram_tensor("wd", [D, DB], BF16, kind="ExternalInput")
    wu_h = nc.dram_tensor("wu", [DB, D], BF16, kind="ExternalInput")
    wr1h_h = nc.dram_tensor("wr1h", [D, E], BF16, kind="ExternalInput")
    wr1l_h = nc.dram_tensor("wr1l", [D, E], BF16, kind="ExternalInput")
    mg_h = nc.dram_tensor("mg", [E, DB], BF16, kind="ExternalInput")
    tbbr_h = nc.dram_tensor("tbbr", [E], F32, kind="ExternalInput")
    colsum_h = nc.dram_tensor("colsum", [E], F32, kind="ExternalInput")
    bsel_h = nc.dram_tensor("bsel", [B], F32, kind="ExternalInput")
    y_h = nc.dram_tensor("y", [TOK, D], F32, kind="ExternalOutput")

    cc_in = nc.dram_tensor("cc_in", [B, 2], F32)
    cc_out = nc.dram_tensor("cc_out", [B, 2], F32, addr_space="Shared")

    DC = D // P     # 8 d-chunks
    NT = TOK // P   # 8 token tiles
    NC512 = TOK // 512  # 2 chunks of 512 tokens
    JM = DB // P    # 2 DB chunks

    with tile_mod.TileContext(nc) as tc:
        with (
            tc.tile_pool(name="big", bufs=1) as big,
            tc.tile_pool(name="consts", bufs=1) as consts,
            tc.tile_pool(name="small", bufs=1) as small,
            tc.tile_pool(name="route", bufs=1) as route,
            tc.tile_pool(name="ysb", bufs=3) as ysb_pool,
            # PSUM budget (8 banks): tr 3 + mm 4 + q 1
            tc.tile_pool(name="psTR", bufs=3, space="PSUM") as psTR,
            tc.tile_pool(name="psMM", bufs=3, space="PSUM") as psMM,
            tc.tile_pool(name="psQ", bufs=2, space="PSUM") as psQ,
        ):
            # ---------------- constants / weights
            ident = consts.tile([P, P], F32)
            make_identity(nc, ident[:])

            eps16 = consts.tile([E, 1], F32)
            nc.vector.memset(eps16[:], EPS)

            wr1h_sb = consts.tile([P, DC, E], BF16)
            nc.sync.dma_start(
                out=wr1h_sb[:], in_=wr1h_h[:, :].rearrange("(dc p) e -> p dc e", p=P)
            )
            wr1l_sb = consts.tile([P, DC, E], BF16)
            nc.sync.dma_start(
                out=wr1l_sb[:], in_=wr1l_h[:, :].rearrange("(dc p) e -> p dc e", p=P)
            )
            mg_sb = consts.tile([E, DB], BF16)
            nc.sync.dma_start(out=mg_sb[:], in_=mg_h[:, :])

            tbbr16 = consts.tile([E, 1], F32)
            nc.sync.dma_start(out=tbbr16[:], in_=tbbr_h[:][:, None])
            colsum16 = consts.tile([E, 1], F32)
            nc.sync.dma_start(out=colsum16[:], in_=colsum_h[:][:, None])
            sel_row = consts.tile([1, B], F32)
            nc.sync.dma_start(out=sel_row[:], in_=bsel_h[:][None, :])
            sel16 = consts.tile([E, B], F32)
            nc.sync.dma_start(
                out=sel16[:],
                in_=bass.AP(tensor=bsel_h, offset=0, ap=[[0, E], [1, B]]),
            )

            # ---------------- load x, stats, transpose to [d, tok] layout
            x_sb = big.tile([P, NT, D], F32)
            xt_bf = big.tile([P, DC, TOK], BF16)
            xlo_bf = big.tile([P, DC, TOK], BF16)
            stats_sb = small.tile([P, NT * 2, 6], F32)

            ci = 0
            for t in range(NT):
                nc.sync.dma_start(
                    out=x_sb[:, t, :], in_=x_h[t * P : (t + 1) * P, :]
                )
                nc.vector.bn_stats(
                    out=stats_sb[:, 2 * t, :], in_=x_sb[:, t, 0:512]
                )
                nc.vector.bn_stats(
                    out=stats_sb[:, 2 * t + 1, :], in_=x_sb[:, t, 512:1024]
                )
                for g in range(DC // 4):   # 4 transposed blocks per psum tile
                    ps_tr = psTR.tile([P, 512], F32, tag="tr")
                    for k in range(4):
                        dc = g * 4 + k
                        nc.tensor.transpose(
                            ps_tr[:, k * P : (k + 1) * P],
                            x_sb[:, t, dc * P : (dc + 1) * P],
                            ident[:],
                        )
                    src_v = ps_tr[:].rearrange("p (k c) -> p k c", k=4)
                    hi = xt_bf[:, g * 4 : (g + 1) * 4, t * P : (t + 1) * P]
                    lo = xlo_bf[:, g * 4 : (g + 1) * 4, t * P : (t + 1) * P]
                    nc.scalar.copy(out=hi, in_=src_v)       # round to bf16
                    nc.vector.tensor_tensor(lo, src_v, hi, ALU.subtract)

            wd_sb = consts.tile([P, DC, DB], BF16)
            nc.sync.dma_start(
                out=wd_sb[:], in_=wd_h[:, :].rearrange("(dc p) j -> p dc j", p=P)
            )
            wu_sb = consts.tile([P, JM, D], BF16)
            nc.sync.dma_start(
                out=wu_sb[:], in_=wu_h[:, :].rearrange("(jc p) d -> p jc d", p=P)
            )

            # per-partition mean/var over this core's shard -> (S1, SS) partials
            mv = small.tile([P, 2], F32)
            nc.vector.bn_aggr(out=mv[:], in_=stats_sb[:])
            s1ss = small.tile([P, 2], F32)
            nc.vector.tensor_scalar_mul(s1ss[:, 0:1], mv[:, 0:1], float(PER_PART))
            msq = small.tile([P, 1], F32)
            nc.vector.tensor_mul(msq[:], mv[:, 0:1], mv[:, 0:1])
            nc.vector.tensor_add(msq[:], msq[:], mv[:, 1:2])
            nc.vector.tensor_scalar_mul(s1ss[:, 1:2], msq[:], float(PER_PART))

            # cross-partition totals + batch scatter, entirely on gpsimd so
            # the collective is not queued behind PE/DVE bulk work
            s_row01 = small.tile([1, 2], F32)
            nc.gpsimd.tensor_reduce(
                s_row01[:], s1ss[:], axis=AX.C, op=ALU.add
            )
            cc_row = small.tile([1, 2 * B], F32)
            nc.gpsimd.tensor_scalar_mul(
                cc_row[:, 0:2], s_row01[:], sel_row[:, 0:1]
            )
            nc.gpsimd.tensor_scalar_mul(
                cc_row[:, 2:4], s_row01[:], sel_row[:, 1:2]
            )
            nc.gpsimd.dma_start(
                out=cc_in[:, :].rearrange("b s -> (b s)")[None, :], in_=cc_row[:]
            )

            # 16-byte cross-core AllReduce of the per-batch (S1, SS)
            nc.gpsimd.collective_compute(
                "AllReduce",
                ALU.add,
                replica_groups=[list(range(NCORES))],
                ins=[cc_in[:, :]],
                outs=[cc_out[:, :]],
            )

            # broadcast the result to 16 partitions and derive per-expert
            # scalars rs16 / cvec16 for the logits affine
            stats4 = small.tile([E, B, 2], F32)
            nc.gpsimd.dma_start(
                out=stats4[:],
                in_=bass.AP(tensor=cc_out, offset=0, ap=[[0, E], [2, B], [1, 2]]),
            )
            tmp2 = small.tile([E, B], F32)
            s1_16 = small.tile([E, 1], F32)
            ss_16 = small.tile([E, 1], F32)
            nc.vector.tensor_mul(tmp2[:], stats4[:, :, 0], sel16[:])
            nc.vector.reduce_sum(s1_16[:], tmp2[:], axis=AX.X)
            nc.vector.tensor_mul(tmp2[:], stats4[:, :, 1], sel16[:])
            nc.vector.reduce_sum(ss_16[:], tmp2[:], axis=AX.X)

            mu16 = small.tile([E, 1], F32)
            var16 = small.tile([E, 1], F32)
            rs16 = small.tile([E, 1], F32)
            cvec16 = small.tile([E, 1], F32)
            nc.vector.tensor_scalar_mul(mu16[:], s1_16[:], 1.0 / NTOT)
            nc.vector.tensor_scalar_mul(var16[:], ss_16[:], 1.0 / NTOT)
            musq = small.tile([E, 1], F32)
            nc.vector.tensor_mul(musq[:], mu16[:], mu16[:])
            nc.vector.tensor_sub(var16[:], var16[:], musq[:])
            nc.scalar.activation(
                out=var16[:], in_=var16[:], func=AF.Sqrt, bias=eps16[:], scale=1.0
            )
            nc.vector.reciprocal(out=rs16[:], in_=var16[:])  # 1/sqrt(var+eps)
            rm16 = small.tile([E, 1], F32)
            nc.vector.tensor_mul(rm16[:], rs16[:], mu16[:])
            nc.vector.tensor_mul(cvec16[:], rm16[:], colsum16[:])
            nc.vector.tensor_sub(cvec16[:], tbbr16[:], cvec16[:])

            # router matmul q^T = Wr1^T x^T via bf16 hi/lo splitting:
            # q = xh@wh + xh@wl + xl@wh reproduces fp32 q to ~1e-5 (no top-2
            # flips), at bf16 matmul speed with tiny stationary loads.
            logitsT = route.tile([E, TOK], F32)
            q_chains = None  # placeholder, chains built below
            for t5 in range(NC512):
                sl = slice(t5 * 512, (t5 + 1) * 512)
                ps_qt = psQ.tile([P, 512], F32, tag="q")
                chains = [(wr1h_sb, xt_bf), (wr1l_sb, xt_bf), (wr1h_sb, xlo_bf)]
                n_mm = len(chains) * DC
                i_mm = 0
                for w_sb, x_src in chains:
                    for dc in range(DC):
                        nc.tensor.matmul(
                            ps_qt[:E, :],
                            w_sb[:, dc, :],
                            x_src[:, dc, sl],
                            start=(i_mm == 0),
                            stop=(i_mm == n_mm - 1),
                        )
                        i_mm += 1
                # logits^T = rs * q^T + cvec  (per-expert scalars, filled in
                # after the collective below; DVE orders by dependency)
                nc.vector.tensor_scalar(
                    out=logitsT[:, sl],
                    in0=ps_qt[:E, :],
                    scalar1=rs16[:],
                    scalar2=cvec16[:],
                    op0=ALU.mult,
                    op1=ALU.add,
                )

            # ---------------- H^T = W_down^T x^T (+gelu)
            gt_sb = big.tile([P, JM, TOK], BF16)
            for t5 in range(NC512):
                sl = slice(t5 * 512, (t5 + 1) * 512)
                for jm in range(JM):
                    ps_h = psMM.tile([P, 512], F32, tag="mm")
                    for dc in range(DC):
                        nc.tensor.matmul(
                            ps_h[:],
                            wd_sb[:, dc, jm * P : (jm + 1) * P],
                            xt_bf[:, dc, sl],
                            start=(dc == 0),
                            stop=(dc == DC - 1),
                        )
                    nc.scalar.activation(
                        out=gt_sb[:, jm, sl], in_=ps_h[:], func=GELU_FUNC
                    )

            # ---------------- routing: transpose logits to token-major, top-2
            logit_n = route.tile([P, NT, E], F32)
            for t in range(NT):
                ps_lt = psTR.tile([P, 512], F32, tag="tr")
                nc.tensor.transpose(
                    ps_lt[:, :E], logitsT[:, t * P : (t + 1) * P], ident[:E, :E]
                )
                nc.scalar.copy(out=logit_n[:, t, :], in_=ps_lt[:, :E])

            m1 = route.tile([P, NT, 1], F32)
            eq1 = route.tile([P, NT, E], F32)
            l2 = route.tile([P, NT, E], F32)
            m2 = route.tile([P, NT, 1], F32)
            eq2 = route.tile([P, NT, E], F32)
            w1 = route.tile([P, NT, 1], F32)
            p_n = route.tile([P, NT, E], F32)

            nc.vector.reduce_max(m1[:], logit_n[:], axis=AX.X)
            nc.vector.tensor_tensor(
                eq1[:], logit_n[:], m1[:].to_broadcast(logit_n.shape), ALU.is_equal
            )
            nc.vector.scalar_tensor_tensor(
                out=l2[:], in0=eq1[:], scalar=-1e30, in1=logit_n[:],
                op0=ALU.mult, op1=ALU.add,
            )
            nc.vector.reduce_max(m2[:], l2[:], axis=AX.X)
            nc.vector.tensor_tensor(
                eq2[:], l2[:], m2[:].to_broadcast(l2.shape), ALU.is_equal
            )
            # w1 = sigmoid(m1 - m2); P = eq2 + w1*(eq1 - eq2)
            nc.vector.tensor_sub(w1[:], m2[:], m1[:])
            nc.scalar.activation(
                out=w1[:], in_=w1[:], func=AF.Sigmoid, scale=-1.0
            )
            nc.vector.tensor_sub(p_n[:], eq1[:], eq2[:])
            nc.vector.tensor_tensor(
                p_n[:], p_n[:], w1[:].to_broadcast(p_n.shape), ALU.mult
            )
            nc.vector.tensor_add(p_n[:], p_n[:], eq2[:])

            pT_sb = route.tile([E, TOK], BF16)
            for t in range(NT):
                ps_pt = psTR.tile([P, P], F32, tag="tr")
                nc.tensor.transpose(ps_pt[:E, :], p_n[:, t, :], ident[:])
                nc.scalar.copy(
                    out=pT_sb[:, t * P : (t + 1) * P], in_=ps_pt[:E, :]
                )

            # ---------------- C^T = Mg^T P^T ; Z^T = G^T * C^T ; y = Z^T^T Wu
            zt_sb = big.tile([P, JM, TOK], BF16)
            for jm in range(JM):
                for t5 in range(NC512):
                    sl = slice(t5 * 512, (t5 + 1) * 512)
                    ps_c = psMM.tile([P, 512], F32, tag="mm")
                    nc.tensor.matmul(
                        ps_c[:],
                        mg_sb[:, jm * P : (jm + 1) * P],
                        pT_sb[:, sl],
                        start=True,
                        stop=True,
                    )
                    nc.vector.tensor_tensor(
                        zt_sb[:, jm, sl], gt_sb[:, jm, sl], ps_c[:], ALU.mult
                    )

            for t in range(NT):
                y_sb = ysb_pool.tile([P, D], F32)
                for dh in range(2):
                    ps_y = psMM.tile([P, 512], F32, tag="mm")
                    for jm in range(JM):
                        nc.tensor.matmul(
                            ps_y[:],
                            zt_sb[:, jm, t * P : (t + 1) * P],
                            wu_sb[:, jm, dh * 512 : (dh + 1) * 512],
                            start=(jm == 0),
                            stop=(jm == JM - 1),
                        )
                    dst = y_sb[:, dh * 512 : (dh + 1) * 512]
                    if dh == 0:
                        nc.scalar.copy(out=dst, in_=ps_y[:])
                    else:
                        nc.vector.tensor_copy(out=dst, in_=ps_y[:])
                nc.sync.dma_start(out=y_h[t * P : (t + 1) * P, :], in_=y_sb[:])

    return nc


_NC_CACHE = {}


def _get_nc():
    if "nc" not in _NC_CACHE:
        _NC_CACHE["nc"] = build_nc()
    return _NC_CACHE["nc"]


def make_in_maps(inputs):
    """Host-side prep: small-tensor precompute + per-core sharding."""
    x = np.ascontiguousarray(np.asarray(inputs["x"], dtype=np.float32))
    task_id = np.asarray(inputs["task_id"])
    task_emb = np.asarray(inputs["task_emb"], dtype=np.float32)
    Wr = np.asarray(inputs["Wr"], dtype=np.float32)
    br = np.asarray(inputs["br"], dtype=np.float32)
    W_down = np.asarray(inputs["W_down"], dtype=np.float32)
    W_up = np.asarray(inputs["W_up"], dtype=np.float32)
    topo_logits = np.asarray(inputs["topo_logits"], dtype=np.float32)

    # gated expert->subspace mask from topo_logits (tiny: [16, 256])
    idx = np.argsort(-topo_logits, axis=1)[:, :RANK_QUOTA]
    mask = np.zeros((E, DB), np.float32)
    np.put_along_axis(mask, idx, 1.0, axis=1)
    tl_vals = np.take_along_axis(topo_logits, idx, axis=1)
    gate = (1.0 / (1.0 + np.exp(-tl_vals))).mean(axis=1)
    mg = np.ascontiguousarray(
        (mask * gate[:, None].astype(np.float32)).astype(ml_dtypes.bfloat16)
    )

    Wr1 = np.ascontiguousarray(Wr[:D])
    tb_br = (task_emb[task_id] @ Wr[D:]) + br          # [B, E]
    colsum = Wr1.sum(axis=0)                            # [E]
    wr1h = Wr1.astype(ml_dtypes.bfloat16)
    wr1l = (Wr1 - wr1h.astype(np.float32)).astype(ml_dtypes.bfloat16)
    wr1h = np.ascontiguousarray(wr1h)
    wr1l = np.ascontiguousarray(wr1l)
    wd_bf = np.ascontiguousarray(W_down.astype(ml_dtypes.bfloat16))
    wu_bf = np.ascontiguousarray(W_up.astype(ml_dtypes.bfloat16))

    xf = x.reshape(B * S, D)
    in_maps = []
    for c in range(NCORES):
        b = c // CORES_PER_BATCH
        t0 = c * TOK
        bsel = np.zeros(B, np.float32)
        bsel[b] = 1.0
        in_maps.append(
            {
                "x": np.ascontiguousarray(xf[t0 : t0 + TOK]),
                "wd": wd_bf,
                "wu": wu_bf,
                "wr1h": wr1h,
                "wr1l": wr1l,
                "mg": mg,
                "tbbr": np.ascontiguousarray(tb_br[b]),
                "colsum": np.ascontiguousarray(colsum),
                "bsel": bsel,
            }
        )
    return in_maps


def run(inputs, trace=False):
    from concourse.bass_utils import run_bass_kernel_spmd

    nc = _get_nc()
    in_maps = make_in_maps(inputs)
    res = run_bass_kernel_spmd(
        nc, in_maps, core_ids=list(range(NCORES)), trace=trace
    )
    y = np.concatenate(
        [res.results[c]["y"] for c in range(NCORES)], axis=0
    ).reshape(B, S, D)
    return y, res


def kernel(**inputs):
    y, _ = run(inputs, trace=False)
    return y


# revision 18
# speedup vs baseline: 1.2720x; 1.1793x over previous
"""Trainium2 Bass kernel for nn_CDSPMoELayer (task-conditioned dual-subspace MoE).

Math reformulation (verified bit-close to the reference on CPU):
  Since Wd[e,r,:] = W_down[:, tl_idx[e,r]] and Wu[e,r,:] = W_up[tl_idx[e,r], :],
  the per-expert low-rank einsums collapse to dense matmuls:
      H = x @ W_down            [N, DB]
      G = gelu_tanh(H)
      C = P @ Mg                [N, DB]   (P: top-2 routing weights scattered
                                           over E; Mg[e,j] = gate[e]*mask[e,j])
      y = (G * C) @ W_up        [N, D]
  The router logits need only two per-batch scalars from the layernorm:
      logits = rs_b * (x @ Wr[:D]) + (tb_br[b] - rs_b*mu_b*colsum(Wr[:D]))

Sharding: data-parallel over tokens. 8 cores; cores 0-3 take batch 0,
cores 4-7 batch 1; each core handles 1024 tokens. Per-batch layernorm
stats (sum, sumsq) are exchanged with a 16-byte AllReduce.

Precision: H/C/y matmuls in bf16 (weights host-cast to bf16); the router
matmul q = x @ Wr1 stays full fp32 so top-2 selections match the reference.
"""

import sys

sys.path.insert(0, "/opt/trn_rl_repo")

import numpy as np
import ml_dtypes

import concourse.bass as bass
import concourse.mybir as mybir
import concourse.tile as tile_mod
import concourse.bass_isa as bass_isa
from concourse.masks import make_identity

# ---------------------------------------------------------------- problem dims
P = 128
B, S, D = 2, 4096, 1024
E, DB, DT = 16, 256, 32
RANK_QUOTA = 64
EPS = 1e-5
NCORES = 8
TOK = B * S // NCORES          # tokens per core = 1024
CORES_PER_BATCH = NCORES // B  # 4
NTOT = S * D                   # stats denominator per batch = 2^22
PER_PART = TOK * D // P        # elements per sbuf partition of the x shard

F32 = mybir.dt.float32
BF16 = mybir.dt.bfloat16

GELU_FUNC = mybir.ActivationFunctionType.Gelu_apprx_tanh


# ------------------------------------------------------- walrus wait workaround
# This container's walrus rejects instructions carrying more than one sem wait
# ("Too many sync wait commands").  Tile's wait assigner can attach several.
# Post-process the serialized BIR: move excess waits onto preceding Drain
# instructions on the same engine, one wait each.
def _split_excess_waits(m):
    n = 0
    for f in m["functions"]:
        blocks = f.get("basicblocks") or f.get("blocks") or []
        for blk in blocks:
            out = []
            for inst in blk["instructions"]:
                si = inst.get("sync_info")
                ow = si.get("on_wait") if si else None
                if ow and len(ow) > 1:
                    for w in ow[:-1]:
                        n += 1
                        out.append(
                            {
                                "debug": inst.get("debug"),
                                "engine": inst["engine"],
                                "ins": [],
                                "outs": [],
                                "name": f"I-wsplit-{n}",
                                "opcode": "Drain",
                                "sync_info": {"on_update": [], "on_wait": [w]},
                            }
                        )
                    si["on_wait"] = [ow[-1]]
                out.append(inst)
            blk["instructions"] = out
    return n


_orig_to_json_bytes = bass.Bass.to_json_bytes


def _patched_to_json_bytes(self):
    import orjson

    raw = _orig_to_json_bytes(self)
    m = orjson.loads(raw)
    if _split_excess_waits(m):
        return orjson.dumps(m)
    return raw


bass.Bass.to_json_bytes = _patched_to_json_bytes


# ------------------------------------------------------------------ the kernel
def build_nc():
    nc = bass.Bass()
    AF = mybir.ActivationFunctionType
    ALU = mybir.AluOpType
    AX = mybir.AxisListType

    x_h = nc.dram_tensor("x", [TOK, D], F32, kind="ExternalInput")
    wd_h = nc.dram_tensor("wd", [D, DB], BF16, kind="ExternalInput")
    wu_h = nc.dram_tensor("wu", [DB, D], BF16, kind="ExternalInput")
    wr1h_h = nc.dram_tensor("wr1h", [D, E], BF16, kind="ExternalInput")
    wr1l_h = nc.dram_tensor("wr1l", [D, E], BF16, kind="ExternalInput")
    mg_h = nc.dram_tensor("mg", [E, DB], BF16, kind="ExternalInput")
    tbbr_h = nc.dram_tensor("tbbr", [E], F32, kind="ExternalInput")
    colsum_h = nc.dram_tensor("colsum", [E], F32, kind="ExternalInput")
    bsel_h = nc.dram_tensor("bsel", [B], F32, kind="ExternalInput")
    y_h = nc.dram_tensor("y", [TOK, D], F32, kind="ExternalOutput")

    cc_in = nc.dram_tensor("cc_in", [B, 2], F32)
    cc_out = nc.dram_tensor("cc_out", [B, 2], F32, addr_space="Shared")

    DC = D // P     # 8 d-chunks
    NT = TOK // P   # 8 token tiles
    NC512 = TOK // 512  # 2 chunks of 512 tokens
    JM = DB // P    # 2 DB chunks

    with tile_mod.TileContext(nc) as tc:
        with (
            tc.tile_pool(name="big", bufs=1) as big,
            tc.tile_pool(name="consts", bufs=1) as consts,
            tc.tile_pool(name="small", bufs=1) as small,
            tc.tile_pool(name="route", bufs=1) as route,
            tc.tile_pool(name="ysb", bufs=3) as ysb_pool,
            # PSUM budget (8 banks): tr 3 + mm 4 + q 1
            tc.tile_pool(name="psTR", bufs=3, space="PSUM") as psTR,
            tc.tile_pool(name="psMM", bufs=3, space="PSUM") as psMM,
            tc.tile_pool(name="psQ", bufs=2, space="PSUM") as psQ,
        ):
            # ---------------- constants / weights
            ident = consts.tile([P, P], F32)
            make_identity(nc, ident[:])

            eps16 = consts.tile([E, 1], F32)
            nc.vector.memset(eps16[:], EPS)

            wr1h_sb = consts.tile([P, DC, E], BF16)
            nc.sync.dma_start(
                out=wr1h_sb[:], in_=wr1h_h[:, :].rearrange("(dc p) e -> p dc e", p=P)
            )
            wr1l_sb = consts.tile([P, DC, E], BF16)
            nc.sync.dma_start(
                out=wr1l_sb[:], in_=wr1l_h[:, :].rearrange("(dc p) e -> p dc e", p=P)
            )
            mg_sb = consts.tile([E, DB], BF16)
            nc.sync.dma_start(out=mg_sb[:], in_=mg_h[:, :])

            tbbr16 = consts.tile([E, 1], F32)
            nc.sync.dma_start(out=tbbr16[:], in_=tbbr_h[:][:, None])
            colsum16 = consts.tile([E, 1], F32)
            nc.sync.dma_start(out=colsum16[:], in_=colsum_h[:][:, None])
            sel_row = consts.tile([1, B], F32)
            nc.sync.dma_start(out=sel_row[:], in_=bsel_h[:][None, :])
            sel16 = consts.tile([E, B], F32)
            nc.sync.dma_start(
                out=sel16[:],
                in_=bass.AP(tensor=bsel_h, offset=0, ap=[[0, E], [1, B]]),
            )

            # ---------------- load x, stats, transpose to [d, tok] layout
            x_sb = big.tile([P, NT, D], F32)
            xt_bf = big.tile([P, DC, TOK], BF16)
            xlo_bf = big.tile([P, DC, TOK], BF16)
            stats_sb = small.tile([P, NT * 2, 6], F32)

            ci = 0
            for t in range(NT):
                nc.sync.dma_start(
                    out=x_sb[:, t, :], in_=x_h[t * P : (t + 1) * P, :]
                )
                with tc.high_priority():
                    # stats feed the cross-core collective: keep them at the
                    # head of the DVE queue, never behind transpose copies
                    nc.vector.bn_stats(
                        out=stats_sb[:, 2 * t, :], in_=x_sb[:, t, 0:512]
                    )
                    nc.vector.bn_stats(
                        out=stats_sb[:, 2 * t + 1, :], in_=x_sb[:, t, 512:1024]
                    )
                for g in range(DC // 4):   # 4 transposed blocks per psum tile
                    ps_tr = psTR.tile([P, 512], F32, tag="tr")
                    for k in range(4):
                        dc = g * 4 + k
                        nc.tensor.transpose(
                            ps_tr[:, k * P : (k + 1) * P],
                            x_sb[:, t, dc * P : (dc + 1) * P],
                            ident[:],
                        )
                    src_v = ps_tr[:].rearrange("p (k c) -> p k c", k=4)
                    hi = xt_bf[:, g * 4 : (g + 1) * 4, t * P : (t + 1) * P]
                    lo = xlo_bf[:, g * 4 : (g + 1) * 4, t * P : (t + 1) * P]
                    nc.scalar.copy(out=hi, in_=src_v)       # round to bf16
                    nc.vector.tensor_tensor(lo, src_v, hi, ALU.subtract)

            wd_sb = consts.tile([P, DC, DB], BF16)
            nc.sync.dma_start(
                out=wd_sb[:], in_=wd_h[:, :].rearrange("(dc p) j -> p dc j", p=P)
            )
            wu_sb = consts.tile([P, JM, D], BF16)
            nc.sync.dma_start(
                out=wu_sb[:], in_=wu_h[:, :].rearrange("(jc p) d -> p jc d", p=P)
            )

            # per-partition mean/var over this core's shard -> (S1, SS) partials
            mv = small.tile([P, 2], F32)
            nc.vector.bn_aggr(out=mv[:], in_=stats_sb[:])
            s1ss = small.tile([P, 2], F32)
            nc.vector.tensor_scalar_mul(s1ss[:, 0:1], mv[:, 0:1], float(PER_PART))
            msq = small.tile([P, 1], F32)
            nc.vector.tensor_mul(msq[:], mv[:, 0:1], mv[:, 0:1])
            nc.vector.tensor_add(msq[:], msq[:], mv[:, 1:2])
            nc.vector.tensor_scalar_mul(s1ss[:, 1:2], msq[:], float(PER_PART))

            # cross-partition totals + batch scatter, entirely on gpsimd so
            # the collective is not queued behind PE/DVE bulk work
            s_row01 = small.tile([1, 2], F32)
            nc.gpsimd.tensor_reduce(
                s_row01[:], s1ss[:], axis=AX.C, op=ALU.add
            )
            cc_row = small.tile([1, 2 * B], F32)
            nc.gpsimd.tensor_scalar_mul(
                cc_row[:, 0:2], s_row01[:], sel_row[:, 0:1]
            )
            nc.gpsimd.tensor_scalar_mul(
                cc_row[:, 2:4], s_row01[:], sel_row[:, 1:2]
            )
            nc.gpsimd.dma_start(
                out=cc_in[:, :].rearrange("b s -> (b s)")[None, :], in_=cc_row[:]
            )

            # 16-byte cross-core AllReduce of the per-batch (S1, SS)
            nc.gpsimd.collective_compute(
                "AllReduce",
                ALU.add,
                replica_groups=[list(range(NCORES))],
                ins=[cc_in[:, :]],
                outs=[cc_out[:, :]],
            )

            # broadcast the result to 16 partitions and derive per-expert
            # scalars rs16 / cvec16 for the logits affine
            stats4 = small.tile([E, B, 2], F32)
            nc.gpsimd.dma_start(
                out=stats4[:],
                in_=bass.AP(tensor=cc_out, offset=0, ap=[[0, E], [2, B], [1, 2]]),
            )
            tmp2 = small.tile([E, B], F32)
            s1_16 = small.tile([E, 1], F32)
            ss_16 = small.tile([E, 1], F32)
            nc.vector.tensor_mul(tmp2[:], stats4[:, :, 0], sel16[:])
            nc.vector.reduce_sum(s1_16[:], tmp2[:], axis=AX.X)
            nc.vector.tensor_mul(tmp2[:], stats4[:, :, 1], sel16[:])
            nc.vector.reduce_sum(ss_16[:], tmp2[:], axis=AX.X)

            mu16 = small.tile([E, 1], F32)
            var16 = small.tile([E, 1], F32)
            rs16 = small.tile([E, 1], F32)
            cvec16 = small.tile([E, 1], F32)
            nc.vector.tensor_scalar_mul(mu16[:], s1_16[:], 1.0 / NTOT)
            nc.vector.tensor_scalar_mul(var16[:], ss_16[:], 1.0 / NTOT)
            musq = small.tile([E, 1], F32)
            nc.vector.tensor_mul(musq[:], mu16[:], mu16[:])
            nc.vector.tensor_sub(var16[:], var16[:], musq[:])
            nc.scalar.activation(
                out=var16[:], in_=var16[:], func=AF.Sqrt, bias=eps16[:], scale=1.0
            )
            nc.vector.reciprocal(out=rs16[:], in_=var16[:])  # 1/sqrt(var+eps)
            rm16 = small.tile([E, 1], F32)
            nc.vector.tensor_mul(rm16[:], rs16[:], mu16[:])
            nc.vector.tensor_mul(cvec16[:], rm16[:], colsum16[:])
            nc.vector.tensor_sub(cvec16[:], tbbr16[:], cvec16[:])

            # router matmul q^T = Wr1^T x^T via bf16 hi/lo splitting:
            # q = xh@wh + xh@wl + xl@wh reproduces fp32 q to ~1e-5 (no top-2
            # flips), at bf16 matmul speed with tiny stationary loads.
            logitsT = route.tile([E, TOK], F32)
            q_chains = None  # placeholder, chains built below
            for t5 in range(NC512):
                sl = slice(t5 * 512, (t5 + 1) * 512)
                ps_qt = psQ.tile([P, 512], F32, tag="q")
                chains = [(wr1h_sb, xt_bf), (wr1l_sb, xt_bf), (wr1h_sb, xlo_bf)]
                n_mm = len(chains) * DC
                i_mm = 0
                for w_sb, x_src in chains:
                    for dc in range(DC):
                        nc.tensor.matmul(
                            ps_qt[:E, :],
                            w_sb[:, dc, :],
                            x_src[:, dc, sl],
                            start=(i_mm == 0),
                            stop=(i_mm == n_mm - 1),
                        )
                        i_mm += 1
                # logits^T = rs * q^T + cvec  (per-expert scalars, filled in
                # after the collective below; DVE orders by dependency)
                nc.vector.tensor_scalar(
                    out=logitsT[:, sl],
                    in0=ps_qt[:E, :],
                    scalar1=rs16[:],
                    scalar2=cvec16[:],
                    op0=ALU.mult,
                    op1=ALU.add,
                )

            # ---------------- H^T = W_down^T x^T (+gelu)
            gt_sb = big.tile([P, JM, TOK], BF16)
            for t5 in range(NC512):
                sl = slice(t5 * 512, (t5 + 1) * 512)
                for jm in range(JM):
                    ps_h = psMM.tile([P, 512], F32, tag="mm")
                    for dc in range(DC):
                        nc.tensor.matmul(
                            ps_h[:],
                            wd_sb[:, dc, jm * P : (jm + 1) * P],
                            xt_bf[:, dc, sl],
                            start=(dc == 0),
                            stop=(dc == DC - 1),
                        )
                    nc.scalar.activation(
                        out=gt_sb[:, jm, sl], in_=ps_h[:], func=GELU_FUNC
                    )

            # ---------------- routing: transpose logits to token-major, top-2
            logit_n = route.tile([P, NT, E], F32)
            for t in range(NT):
                ps_lt = psTR.tile([P, 512], F32, tag="tr")
                nc.tensor.transpose(
                    ps_lt[:, :E], logitsT[:, t * P : (t + 1) * P], ident[:E, :E]
                )
                nc.scalar.copy(out=logit_n[:, t, :], in_=ps_lt[:, :E])

            m1 = route.tile([P, NT, 1], F32)
            eq1 = route.tile([P, NT, E], F32)
            l2 = route.tile([P, NT, E], F32)
            m2 = route.tile([P, NT, 1], F32)
            eq2 = route.tile([P, NT, E], F32)
            w1 = route.tile([P, NT, 1], F32)
            p_n = route.tile([P, NT, E], F32)

            nc.vector.reduce_max(m1[:], logit_n[:], axis=AX.X)
            nc.vector.tensor_tensor(
                eq1[:], logit_n[:], m1[:].to_broadcast(logit_n.shape), ALU.is_equal
            )
            nc.vector.scalar_tensor_tensor(
                out=l2[:], in0=eq1[:], scalar=-1e30, in1=logit_n[:],
                op0=ALU.mult, op1=ALU.add,
            )
            nc.vector.reduce_max(m2[:], l2[:], axis=AX.X)
            nc.vector.tensor_tensor(
                eq2[:], l2[:], m2[:].to_broadcast(l2.shape), ALU.is_equal
            )
            # w1 = sigmoid(m1 - m2); P = eq2 + w1*(eq1 - eq2)
            nc.vector.tensor_sub(w1[:], m2[:], m1[:])
            nc.scalar.activation(
                out=w1[:], in_=w1[:], func=AF.Sigmoid, scale=-1.0
            )
            nc.vector.tensor_sub(p_n[:], eq1[:], eq2[:])
            nc.vector.tensor_tensor(
                p_n[:], p_n[:], w1[:].to_broadcast(p_n.shape), ALU.mult
            )
            nc.vector.tensor_add(p_n[:], p_n[:], eq2[:])

            pT_sb = route.tile([E, TOK], BF16)
            for t in range(NT):
                ps_pt = psTR.tile([P, P], F32, tag="tr")
                nc.tensor.transpose(ps_pt[:E, :], p_n[:, t, :], ident[:])
                nc.scalar.copy(
                    out=pT_sb[:, t * P : (t + 1) * P], in_=ps_pt[:E, :]
                )

            # ---------------- C^T = Mg^T P^T ; Z^T = G^T * C^T ; y = Z^T^T Wu
            zt_sb = big.tile([P, JM, TOK], BF16)
            for jm in range(JM):
                for t5 in range(NC512):
                    sl = slice(t5 * 512, (t5 + 1) * 512)
                    ps_c = psMM.tile([P, 512], F32, tag="mm")
                    nc.tensor.matmul(
                        ps_c[:],
                        mg_sb[:, jm * P : (jm + 1) * P],
                        pT_sb[:, sl],
                        start=True,
                        stop=True,
                    )
                    nc.vector.tensor_tensor(
                        zt_sb[:, jm, sl], gt_sb[:, jm, sl], ps_c[:], ALU.mult
                    )

            for t in range(NT):
                y_sb = ysb_pool.tile([P, D], F32)
                for dh in range(2):
                    ps_y = psMM.tile([P, 512], F32, tag="mm")
                    for jm in range(JM):
                        nc.tensor.matmul(
                            ps_y[:],
                            zt_sb[:, jm, t * P : (t + 1) * P],
                            wu_sb[:, jm, dh * 512 : (dh + 1) * 512],
                            start=(jm == 0),
                            stop=(jm == JM - 1),
                        )
                    dst = y_sb[:, dh * 512 : (dh + 1) * 512]
                    if dh == 0:
                        nc.scalar.copy(out=dst, in_=ps_y[:])
                    else:
                        nc.vector.tensor_copy(out=dst, in_=ps_y[:])
                nc.sync.dma_start(out=y_h[t * P : (t + 1) * P, :], in_=y_sb[:])

    return nc


_NC_CACHE = {}


def _get_nc():
    if "nc" not in _NC_CACHE:
        _NC_CACHE["nc"] = build_nc()
    return _NC_CACHE["nc"]


def make_in_maps(inputs):
    """Host-side prep: small-tensor precompute + per-core sharding."""
    x = np.ascontiguousarray(np.asarray(inputs["x"], dtype=np.float32))
    task_id = np.asarray(inputs["task_id"])
    task_emb = np.asarray(inputs["task_emb"], dtype=np.float32)
    Wr = np.asarray(inputs["Wr"], dtype=np.float32)
    br = np.asarray(inputs["br"], dtype=np.float32)
    W_down = np.asarray(inputs["W_down"], dtype=np.float32)
    W_up = np.asarray(inputs["W_up"], dtype=np.float32)
    topo_logits = np.asarray(inputs["topo_logits"], dtype=np.float32)

    # gated expert->subspace mask from topo_logits (tiny: [16, 256])
    idx = np.argsort(-topo_logits, axis=1)[:, :RANK_QUOTA]
    mask = np.zeros((E, DB), np.float32)
    np.put_along_axis(mask, idx, 1.0, axis=1)
    tl_vals = np.take_along_axis(topo_logits, idx, axis=1)
    gate = (1.0 / (1.0 + np.exp(-tl_vals))).mean(axis=1)
    mg = np.ascontiguousarray(
        (mask * gate[:, None].astype(np.float32)).astype(ml_dtypes.bfloat16)
    )

    Wr1 = np.ascontiguousarray(Wr[:D])
    tb_br = (task_emb[task_id] @ Wr[D:]) + br          # [B, E]
    colsum = Wr1.sum(axis=0)                            # [E]
    wr1h = Wr1.astype(ml_dtypes.bfloat16)
    wr1l = (Wr1 - wr1h.astype(np.float32)).astype(ml_dtypes.bfloat16)
    wr1h = np.ascontiguousarray(wr1h)
    wr1l = np.ascontiguousarray(wr1l)
    wd_bf = np.ascontiguousarray(W_down.astype(ml_dtypes.bfloat16))
    wu_bf = np.ascontiguousarray(W_up.astype(ml_dtypes.bfloat16))

    xf = x.reshape(B * S, D)
    in_maps = []
    for c in range(NCORES):
        b = c // CORES_PER_BATCH
        t0 = c * TOK
        bsel = np.zeros(B, np.float32)
        bsel[b] = 1.0
        in_maps.append(
            {
                "x": np.ascontiguousarray(xf[t0 : t0 + TOK]),
                "wd": wd_bf,
                "wu": wu_bf,
                "wr1h": wr1h,
                "wr1l": wr1l,
                "mg": mg,
                "tbbr": np.ascontiguousarray(tb_br[b]),
                "colsum": np.ascontiguousarray(colsum),
                "bsel": bsel,
            }
        )
    return in_maps


def run(inputs, trace=False):
    from concourse.bass_utils import run_bass_kernel_spmd

    nc = _get_nc()
    in_maps = make_in_maps(inputs)
    res = run_bass_kernel_spmd(
        nc, in_maps, core_ids=list(range(NCORES)), trace=trace
    )
    y = np.concatenate(
        [res.results[c]["y"] for c in range(NCORES)], axis=0
    ).reshape(B, S, D)
    return y, res


def kernel(**inputs):
    y, _ = run(inputs, trace=False)
    return y
